# revision 18
# baseline (speedup 1.0000x reference)
"""GPT2-style fused attention (DecisionTransformer) on 8 Trainium2 NeuronCores.

Sharding: tensor-parallel over the 16 heads (2 heads per core, both batch
elements on every core).  Each core:
  - loads the full hidden_states [4096, 1024],
  - computes Q/K/V for its 2 heads (transposed layout via PE transposes),
  - causal attention for its 4 (batch, head) pairs: scores^T = K @ Q^T,
    exp (no max subtraction -- logits are small and bounded), ones-column
    appended to V gives the softmax denominator for free in the A@V matmul,
  - row-parallel output projection with its 128 rows of c_proj_w,
  - writes a full-shape partial output [4096, 1024].
Host gathers with a sum over the 8 partials (the row-parallel all-reduce)
and adds c_proj_b.

Matmuls run in float32r (full-rate fp32 streaming, ~tf32-like rounding);
measured output error vs the fp32 reference is ~2e-4 relative to absmax.
"""

import sys

for _p in ("/opt/trn_rl_repo",):
    if _p not in sys.path:
        sys.path.insert(0, _p)

import numpy as np

import concourse.bass as bass
import concourse.mybir as mybir
import concourse.tile as tile
from concourse import bacc
from concourse.bass_utils import run_bass_kernel_spmd
from concourse.masks import make_identity

P = 128
B, S, D, H, HD = 2, 2048, 1024, 16, 64
T = B * S              # 4096 tokens
FQKV = 3 * P           # 384 per-core qkv features (q128 | k128 | v128)
KO = D // P            # 8 contraction chunks
TCH = 512              # token chunk for qkv phase
NTCH = T // TCH        # 8
QC = 512               # query chunk in attention
NQC = S // QC          # 4
NKB = S // P           # 16 key blocks per sequence
SCALE = 1.0 / float(HD) ** 0.5
N_CORES = 8
HPC = H // N_CORES     # 2 heads per core

f32 = mybir.dt.float32
f32r = mybir.dt.float32r
MM_DT = f32r


def _emit_body(nc, tc, pools, consts, it):
    (xin_pool, xt_pool, qkvt_pool, vaug_pool, pt_pool, atn_pool, out_pool,
     small_pool, ps_mm, ps_s, ps_o) = pools
    (wqkv_sb, wp_sb, bqkv_sb, ident_f32, ident2, mask128,
     x_d, out_d) = consts

    qkvt = [
        qkvt_pool.tile([P, T], MM_DT, tag=f"qkvt{c}", name=f"qkvt{c}")
        for c in range(3)
    ]
    vaug = [
        vaug_pool.tile([P, NKB, HD + 1], MM_DT, tag=f"vaug{p}", name=f"vaug{p}")
        for p in range(B * HPC)
    ]
    atn = [
        atn_pool.tile([P, NQC, QC], MM_DT, tag=f"atn{b}", name=f"atn{b}")
        for b in range(B)
    ]

    # ---- phase 1+2: X^T tiles and QKV projection, per token chunk ----
    for i in range(NTCH):
        xt = xt_pool.tile([P, KO, TCH], MM_DT, tag="xt", name="xt")
        xins = []
        for j in range(TCH // P):
            xi = xin_pool.tile([P, D], f32, tag="xi", name="xi")
            nc.sync.dma_start(
                xi[:], x_d[i * TCH + j * P : i * TCH + (j + 1) * P, :]
            )
            xins.append(xi)
        # 4 PE transposes share one PSUM bank -> single wide eviction
        for ko in range(KO):
            ps = ps_mm.tile([P, TCH], f32, tag="mm", name="psmm")
            for j in range(TCH // P):
                nc.tensor.transpose(
                    ps[:, j * P : (j + 1) * P],
                    xins[j][:, ko * P : (ko + 1) * P],
                    ident_f32[:],
                )
            nc.vector.tensor_copy(xt[:, ko, :], ps[:])
        for fc in range(3):
            ps = ps_mm.tile([P, TCH], f32, tag="mm", name="psmm")
            for ko in range(KO):
                nc.tensor.matmul(
                    ps[:],
                    wqkv_sb[:, ko, fc * P : (fc + 1) * P],
                    xt[:, ko, :],
                    start=(ko == 0),
                    stop=(ko == KO - 1),
                )
            # evict + per-partition bias add on ScalarE
            nc.scalar.activation(
                qkvt[fc][:, i * TCH : (i + 1) * TCH],
                ps[:],
                mybir.ActivationFunctionType.Identity,
                bias=bqkv_sb[:, fc : fc + 1],
            )

    # ---- phase 3: V_aug (V back to natural layout + ones column) ----
    for p in range(B * HPC):
        b, hl = p // HPC, p % HPC
        vt = qkvt[2][hl * HD : (hl + 1) * HD, b * S : (b + 1) * S]
        nc.vector.memset(vaug[p][:, :, HD : HD + 1].bitcast(f32), 1.0)
        for kb in range(NKB):
            ps = ps_mm.tile([P, TCH], f32, tag="mm", name="psmm")
            nc.tensor.transpose(
                ps[:, :HD].bitcast(f32r),
                vt[:, kb * P : (kb + 1) * P],
                ident2[hl * HD : (hl + 1) * HD, :],
            )
            nc.vector.tensor_copy(vaug[p][:, kb, :HD], ps[:, :HD])

    # ---- phase 4+5: attention + output projection ----
    for b in range(B):
        for qc in range(NQC):
            for hl in range(HPC):
                p = b * HPC + hl
                qt = qkvt[0][hl * HD : (hl + 1) * HD, b * S : (b + 1) * S]
                kt = qkvt[1][hl * HD : (hl + 1) * HD, b * S : (b + 1) * S]
                rhs_q = qt[:, qc * QC : (qc + 1) * QC]
                po = ps_o.tile([HD + 1, QC], f32, tag="po", name="pso")
                nkb = (qc + 1) * (QC // P)
                for kb in range(nkb):
                    ps = ps_s.tile([P, QC], f32, tag="s", name="pss")
                    nc.tensor.matmul(
                        ps[:],
                        kt[:, kb * P : (kb + 1) * P],
                        rhs_q,
                        start=True,
                        stop=True,
                    )
                    pt = pt_pool.tile([P, QC], MM_DT, tag="pt", name="pt")
                    j = kb - qc * (QC // P)
                    if j < 0:
                        nc.scalar.activation(
                            pt[:],
                            ps[:],
                            mybir.ActivationFunctionType.Exp,
                            scale=SCALE,
                        )
                    else:
                        # diagonal block: cols < j*128 fully masked
                        if j > 0:
                            nc.gpsimd.memset(pt[:, : j * P].bitcast(f32), 0.0)
                        nc.scalar.activation(
                            pt[:, j * P :],
                            ps[:, j * P :],
                            mybir.ActivationFunctionType.Exp,
                            scale=SCALE,
                        )
                        nc.gpsimd.tensor_tensor(
                            pt[:, j * P : (j + 1) * P],
                            pt[:, j * P : (j + 1) * P],
                            mask128[:],
                            mybir.AluOpType.mult,
                        )
                    nc.tensor.matmul(
                        po[:],
                        vaug[p][:, kb, :],
                        pt[:],
                        start=(kb == 0),
                        stop=(kb == nkb - 1),
                    )
                # normalize: A^T = O^T_u * (1/denom), denom = po[64]
                rec = small_pool.tile([1, QC], f32, tag="rec", name="rec")
                nc.vector.reciprocal(rec[:], po[HD : HD + 1, :])
                rbc = small_pool.tile([HD, QC], f32, tag="rbc", name="rbc")
                nc.gpsimd.partition_broadcast(rbc[:], rec[:])
                nc.vector.tensor_tensor(
                    atn[b][hl * HD : (hl + 1) * HD, qc, :],
                    po[:HD, :],
                    rbc[:],
                    mybir.AluOpType.mult,
                )
            # output projection for this (b, qc)
            for qb in range(QC // P):
                for nck in range(2):
                    pp = ps_mm.tile([P, TCH], f32, tag="mm", name="psmm")
                    nc.tensor.matmul(
                        pp[:, :512],
                        atn[b][:, qc, qb * P : (qb + 1) * P],
                        wp_sb[:, nck * 512 : (nck + 1) * 512],
                        start=True,
                        stop=True,
                    )
                    ot = out_pool.tile([P, 512], f32, tag="ot", name="ot")
                    nc.vector.tensor_copy(ot[:], pp[:, :512])
                    row = b * S + qc * QC + qb * P
                    nc.sync.dma_start(
                        out_d[row : row + P, nck * 512 : (nck + 1) * 512],
                        ot[:],
                    )


def _build_program(iters=1):
    nc = bacc.Bacc(None, target_bir_lowering=False)

    x_d = nc.dram_tensor("x", [T, D], f32, kind="ExternalInput")
    wqkv_d = nc.dram_tensor("w_qkv", [D, FQKV], f32, kind="ExternalInput")
    bqkv_d = nc.dram_tensor("b_qkv", [FQKV], f32, kind="ExternalInput")
    wp_d = nc.dram_tensor("w_proj", [P, D], f32, kind="ExternalInput")
    out_d = nc.dram_tensor("out", [T, D], f32, kind="ExternalOutput")

    with tile.TileContext(nc) as tc:
        with (
            tc.tile_pool(name="const", bufs=1) as const,
            tc.tile_pool(name="xin", bufs=6) as xin_pool,
            tc.tile_pool(name="xt", bufs=2) as xt_pool,
            tc.tile_pool(name="qkvt", bufs=1) as qkvt_pool,
            tc.tile_pool(name="vaug", bufs=1) as vaug_pool,
            tc.tile_pool(name="pt", bufs=4) as pt_pool,
            tc.tile_pool(name="atn", bufs=1) as atn_pool,
            tc.tile_pool(name="outp", bufs=4) as out_pool,
            tc.tile_pool(name="small", bufs=3) as small_pool,
            tc.tile_pool(name="ps_mm", bufs=3, space="PSUM") as ps_mm,
            tc.tile_pool(name="ps_s", bufs=3, space="PSUM") as ps_s,
            tc.tile_pool(name="ps_o", bufs=2, space="PSUM") as ps_o,
        ):
            # ---- constants ----
            # weights: gpsimd "casting" DMA fp32 -> f32r (bit-identical move;
            # satisfies the BIR fp32r-producer rule)
            wqkv_sb = const.tile([P, KO, FQKV], MM_DT)
            nc.gpsimd.dma_start(
                wqkv_sb[:], wqkv_d.rearrange("(ko p) f -> p ko f", p=P)
            )
            wp_sb = const.tile([P, D], MM_DT)
            nc.gpsimd.dma_start(wp_sb[:], wp_d[:])
            bqkv_sb = const.tile([P, 3], f32)
            nc.sync.dma_start(bqkv_sb[:], bqkv_d.rearrange("(c p) -> p c", p=P))
            ident_f32 = const.tile([P, P], f32)
            make_identity(nc, ident_f32[:])
            # ident2[r, c] = 1 iff r == c or r == c + 64 (c < 64): slices
            # [:64] / [64:] are 64x64 identities at partition base 0 / 64,
            # for transposing the per-head V^T chunks (lhsT and rhs of a
            # matmul must share the same base partition).
            ident2_f32 = const.tile([P, HD], f32)
            nc.gpsimd.memset(ident2_f32[:], 0.0)
            for base in (0, -HD):
                nc.gpsimd.affine_select(
                    out=ident2_f32[:],
                    in_=ident2_f32[:],
                    compare_op=mybir.AluOpType.not_equal,
                    fill=1.0,
                    base=base,
                    pattern=[[-1, HD]],
                    channel_multiplier=1,
                )
            ident2 = const.tile([P, HD], MM_DT)
            nc.vector.tensor_copy(ident2[:], ident2_f32[:])
            # mask128[k, q] = 1.0 if k <= q else 0.0
            mask128 = const.tile([P, P], f32)
            nc.gpsimd.memset(mask128[:], 1.0)
            nc.gpsimd.affine_select(
                out=mask128[:],
                in_=mask128[:],
                compare_op=mybir.AluOpType.is_ge,
                fill=0.0,
                base=0,
                pattern=[[1, P]],
                channel_multiplier=-1,
            )

            pools = (xin_pool, xt_pool, qkvt_pool, vaug_pool, pt_pool,
                     atn_pool, out_pool, small_pool, ps_mm, ps_s, ps_o)
            consts = (wqkv_sb, wp_sb, bqkv_sb, ident_f32, ident2, mask128,
                      x_d, out_d)
            for it in range(iters):
                _emit_body(nc, tc, pools, consts, it)

    nc.compile()
    return nc


_CACHE = {}


def get_program(iters=1):
    if iters not in _CACHE:
        _CACHE[iters] = _build_program(iters)
    return _CACHE[iters]


def make_in_maps(hidden_states, c_attn_w, c_attn_b, c_proj_w):
    x = np.ascontiguousarray(
        np.asarray(hidden_states, dtype=np.float32).reshape(T, D)
    )
    wa = np.asarray(c_attn_w, dtype=np.float32)
    ba = np.asarray(c_attn_b, dtype=np.float32)
    wp = np.asarray(c_proj_w, dtype=np.float32)
    in_maps = []
    for c in range(N_CORES):
        lo, hi = c * P, (c + 1) * P
        w_qkv = np.ascontiguousarray(
            np.concatenate(
                [wa[:, lo:hi], wa[:, D + lo : D + hi], wa[:, 2 * D + lo : 2 * D + hi]],
                axis=1,
            )
        )
        b_qkv = np.ascontiguousarray(
            np.concatenate([ba[lo:hi], ba[D + lo : D + hi], ba[2 * D + lo : 2 * D + hi]])
        )
        w_proj = np.ascontiguousarray(wp[lo:hi, :])
        in_maps.append({"x": x, "w_qkv": w_qkv, "b_qkv": b_qkv, "w_proj": w_proj})
    return in_maps


def kernel(hidden_states, c_attn_w, c_attn_b, c_proj_w, c_proj_b):
    nc = get_program()
    in_maps = make_in_maps(hidden_states, c_attn_w, c_attn_b, c_proj_w)
    res = run_bass_kernel_spmd(nc, in_maps, list(range(N_CORES)))
    # unshard: row-parallel projection partials sum + bias
    acc = res.results[0]["out"]
    for c in range(1, N_CORES):
        acc = acc + res.results[c]["out"]
    acc = acc + np.asarray(c_proj_b, dtype=np.float32)[None, :]
    return acc.reshape(B, S, D).astype(np.float32)


if __name__ == "__main__":
    rng = np.random.default_rng(0)
    hs = rng.standard_normal((B, S, D), dtype=np.float32)
    wa = rng.standard_normal((D, 3 * D), dtype=np.float32) * 0.02
    ba = rng.standard_normal((3 * D,), dtype=np.float32) * 0.02
    wp = rng.standard_normal((D, D), dtype=np.float32) * 0.02
    bp = rng.standard_normal((D,), dtype=np.float32) * 0.02
    out = kernel(hs, wa, ba, wp, bp)
    print("out", out.shape, out.dtype, float(np.abs(out).max()))


# revision 23
# speedup vs baseline: 17.7058x; 17.7058x over previous
"""GPT2-style fused attention (DecisionTransformer) on 8 Trainium2 NeuronCores.

Sharding: tensor-parallel over the 16 heads (2 heads per core, both batch
elements on every core).  Each core:
  - loads the full hidden_states [4096, 1024],
  - computes Q/K/V for its 2 heads (transposed layout via PE transposes),
  - causal attention for its 4 (batch, head) pairs: scores^T = K @ Q^T,
    exp (no max subtraction -- logits are small and bounded), ones-column
    appended to V gives the softmax denominator for free in the A@V matmul,
  - row-parallel output projection with its 128 rows of c_proj_w,
  - writes a full-shape partial output [4096, 1024].
Host gathers with a sum over the 8 partials (the row-parallel all-reduce)
and adds c_proj_b.

Matmuls run in float32r (full-rate fp32 streaming, ~tf32-like rounding);
measured output error vs the fp32 reference is ~2e-4 relative to absmax.
"""

import sys

for _p in ("/opt/trn_rl_repo",):
    if _p not in sys.path:
        sys.path.insert(0, _p)

import numpy as np

import concourse.bass as bass
import concourse.mybir as mybir
import concourse.tile as tile
from concourse import bacc
from concourse.bass_utils import run_bass_kernel_spmd
from concourse.masks import make_identity

P = 128
B, S, D, H, HD = 2, 2048, 1024, 16, 64
T = B * S              # 4096 tokens
FQKV = 3 * P           # 384 per-core qkv features (q128 | k128 | v128)
KO = D // P            # 8 contraction chunks
TCH = 512              # token chunk for qkv phase
NTCH = T // TCH        # 8
QC = 512               # query chunk in attention
NQC = S // QC          # 4
NKB = S // P           # 16 key blocks per sequence
SCALE = 1.0 / float(HD) ** 0.5
N_CORES = 8
HPC = H // N_CORES     # 2 heads per core

f32 = mybir.dt.float32
f32r = mybir.dt.float32r
MM_DT = f32r


def _emit_body(nc, tc, pools, consts, it):
    (xin_pool, xt_pool, qkvt_pool, vaug_pool, pt_pool, atn_pool, out_pool,
     small_pool, ps_mm, ps_s, ps_o) = pools
    (wqkv_sb, wp_sb, bqkv_sb, ident_f32, ident2, mask128,
     ones1, x_d, out_d) = consts

    qkvt = [
        qkvt_pool.tile([P, T], MM_DT, tag=f"qkvt{c}", name=f"qkvt{c}")
        for c in range(3)
    ]
    vaug = [
        vaug_pool.tile([P, NKB, HD + 1], MM_DT, tag=f"vaug{p}", name=f"vaug{p}")
        for p in range(B * HPC)
    ]
    atn = [
        atn_pool.tile([P, NQC, QC], MM_DT, tag=f"atn{b}", name=f"atn{b}")
        for b in range(B)
    ]

    # ---- phase 1+2: X^T tiles and QKV projection, per token chunk ----
    for i in range(NTCH):
        xt = xt_pool.tile([P, KO, TCH], MM_DT, tag="xt", name="xt")
        xins = []
        for j in range(TCH // P):
            xi = xin_pool.tile([P, D], f32, tag="xi", name="xi")
            nc.sync.dma_start(
                xi[:], x_d[i * TCH + j * P : i * TCH + (j + 1) * P, :]
            )
            xins.append(xi)
        # 4 PE transposes share one PSUM bank -> single wide eviction
        for ko in range(KO):
            ps = ps_mm.tile([P, TCH], f32, tag="mm", name="psmm")
            for j in range(TCH // P):
                nc.tensor.transpose(
                    ps[:, j * P : (j + 1) * P],
                    xins[j][:, ko * P : (ko + 1) * P],
                    ident_f32[:],
                )
            nc.vector.tensor_copy(xt[:, ko, :], ps[:])
        for fc in range(3):
            ps = ps_mm.tile([P, TCH], f32, tag="mm", name="psmm")
            for ko in range(KO):
                nc.tensor.matmul(
                    ps[:],
                    wqkv_sb[:, ko, fc * P : (fc + 1) * P],
                    xt[:, ko, :],
                    start=(ko == 0),
                    stop=(ko == KO - 1),
                )
            # evict + per-partition bias add on ScalarE
            nc.scalar.activation(
                qkvt[fc][:, i * TCH : (i + 1) * TCH],
                ps[:],
                mybir.ActivationFunctionType.Identity,
                bias=bqkv_sb[:, fc : fc + 1],
            )

    # ---- phase 3: V_aug (V back to natural layout + ones column) ----
    for p in range(B * HPC):
        b, hl = p // HPC, p % HPC
        vt = qkvt[2][hl * HD : (hl + 1) * HD, b * S : (b + 1) * S]
        nc.vector.memset(vaug[p][:, :, HD : HD + 1].bitcast(f32), 1.0)
        for kb in range(NKB):
            ps = ps_mm.tile([P, TCH], f32, tag="mm", name="psmm")
            nc.tensor.transpose(
                ps[:, :HD].bitcast(f32r),
                vt[:, kb * P : (kb + 1) * P],
                ident2[hl * HD : (hl + 1) * HD, :],
            )
            nc.vector.tensor_copy(vaug[p][:, kb, :HD], ps[:, :HD])

    # ---- phase 4+5: attention + output projection ----
    for b in range(B):
        for qc in range(NQC):
            for hl in range(HPC):
                p = b * HPC + hl
                qt = qkvt[0][hl * HD : (hl + 1) * HD, b * S : (b + 1) * S]
                kt = qkvt[1][hl * HD : (hl + 1) * HD, b * S : (b + 1) * S]
                rhs_q = qt[:, qc * QC : (qc + 1) * QC]
                po = ps_o.tile([HD + 1, QC], f32, tag="po", name="pso")
                nkb = (qc + 1) * (QC // P)
                for kb in range(nkb):
                    ps = ps_s.tile([P, QC], f32, tag="s", name="pss")
                    nc.tensor.matmul(
                        ps[:],
                        kt[:, kb * P : (kb + 1) * P],
                        rhs_q,
                        start=True,
                        stop=True,
                    )
                    pt = pt_pool.tile([P, QC], MM_DT, tag="pt", name="pt")
                    j = kb - qc * (QC // P)
                    if j < 0:
                        nc.scalar.activation(
                            pt[:],
                            ps[:],
                            mybir.ActivationFunctionType.Exp,
                            scale=SCALE,
                        )
                    else:
                        # diagonal block: cols < j*128 fully masked
                        if j > 0:
                            nc.vector.memset(pt[:, : j * P].bitcast(f32), 0.0)
                        nc.scalar.activation(
                            pt[:, j * P :],
                            ps[:, j * P :],
                            mybir.ActivationFunctionType.Exp,
                            scale=SCALE,
                        )
                        nc.vector.tensor_tensor(
                            pt[:, j * P : (j + 1) * P],
                            pt[:, j * P : (j + 1) * P],
                            mask128[:],
                            mybir.AluOpType.mult,
                        )
                    nc.tensor.matmul(
                        po[:],
                        vaug[p][:, kb, :],
                        pt[:],
                        start=(kb == 0),
                        stop=(kb == nkb - 1),
                    )
                # normalize: A^T = O^T_u * (1/denom), denom = po[64]
                rec = small_pool.tile([1, QC], MM_DT, tag="rec", name="rec")
                with nc.allow_low_precision(reason="f32r rounding for PE bcast"):
                    nc.vector.reciprocal(rec[:], po[HD : HD + 1, :])
                rbc = ps_o.tile([HD, QC], f32, tag="po", name="pso")
                nc.tensor.matmul(
                    rbc[:],
                    ones1[:, :HD],
                    rec[:],
                    start=True,
                    stop=True,
                )
                rbs = small_pool.tile([HD, QC], f32, tag="rbs", name="rbs")
                nc.vector.tensor_copy(rbs[:], rbc[:])
                nc.vector.tensor_tensor(
                    atn[b][hl * HD : (hl + 1) * HD, qc, :],
                    po[:HD, :],
                    rbs[:],
                    mybir.AluOpType.mult,
                )
            # output projection for this (b, qc)
            for qb in range(QC // P):
                for nck in range(2):
                    pp = ps_mm.tile([P, TCH], f32, tag="mm", name="psmm")
                    nc.tensor.matmul(
                        pp[:, :512],
                        atn[b][:, qc, qb * P : (qb + 1) * P],
                        wp_sb[:, nck * 512 : (nck + 1) * 512],
                        start=True,
                        stop=True,
                    )
                    ot = out_pool.tile([P, 512], f32, tag="ot", name="ot")
                    nc.vector.tensor_copy(ot[:], pp[:, :512])
                    row = b * S + qc * QC + qb * P
                    nc.sync.dma_start(
                        out_d[row : row + P, nck * 512 : (nck + 1) * 512],
                        ot[:],
                    )


def _build_program(iters=1):
    nc = bacc.Bacc(None, target_bir_lowering=False)

    x_d = nc.dram_tensor("x", [T, D], f32, kind="ExternalInput")
    wqkv_d = nc.dram_tensor("w_qkv", [D, FQKV], f32, kind="ExternalInput")
    bqkv_d = nc.dram_tensor("b_qkv", [FQKV], f32, kind="ExternalInput")
    wp_d = nc.dram_tensor("w_proj", [P, D], f32, kind="ExternalInput")
    out_d = nc.dram_tensor("out", [T, D], f32, kind="ExternalOutput")

    with tile.TileContext(nc) as tc:
        with (
            tc.tile_pool(name="const", bufs=1) as const,
            tc.tile_pool(name="xin", bufs=6) as xin_pool,
            tc.tile_pool(name="xt", bufs=2) as xt_pool,
            tc.tile_pool(name="qkvt", bufs=1) as qkvt_pool,
            tc.tile_pool(name="vaug", bufs=1) as vaug_pool,
            tc.tile_pool(name="pt", bufs=4) as pt_pool,
            tc.tile_pool(name="atn", bufs=1) as atn_pool,
            tc.tile_pool(name="outp", bufs=4) as out_pool,
            tc.tile_pool(name="small", bufs=3) as small_pool,
            tc.tile_pool(name="ps_mm", bufs=3, space="PSUM") as ps_mm,
            tc.tile_pool(name="ps_s", bufs=3, space="PSUM") as ps_s,
            tc.tile_pool(name="ps_o", bufs=2, space="PSUM") as ps_o,
        ):
            # ---- constants ----
            # weights: gpsimd "casting" DMA fp32 -> f32r (bit-identical move;
            # satisfies the BIR fp32r-producer rule)
            wqkv_sb = const.tile([P, KO, FQKV], MM_DT)
            wq_stage = xt_pool.tile([P, KO, FQKV], f32, tag="xt", name="xt")
            nc.sync.dma_start(
                wq_stage[:], wqkv_d.rearrange("(ko p) f -> p ko f", p=P)
            )
            nc.vector.tensor_copy(wqkv_sb[:], wq_stage[:])
            wp_sb = const.tile([P, D], MM_DT)
            wp_stage = xin_pool.tile([P, D], f32, tag="xi", name="xi")
            nc.sync.dma_start(wp_stage[:], wp_d[:])
            nc.vector.tensor_copy(wp_sb[:], wp_stage[:])
            bqkv_sb = const.tile([P, 3], f32)
            nc.sync.dma_start(bqkv_sb[:], bqkv_d.rearrange("(c p) -> p c", p=P))
            ident_f32 = const.tile([P, P], f32)
            make_identity(nc, ident_f32[:])
            # ident2[r, c] = 1 iff r == c or r == c + 64 (c < 64): slices
            # [:64] / [64:] are 64x64 identities at partition base 0 / 64,
            # for transposing the per-head V^T chunks (lhsT and rhs of a
            # matmul must share the same base partition).
            ident2_f32 = const.tile([P, HD], f32)
            nc.gpsimd.memset(ident2_f32[:], 0.0)
            for base in (0, -HD):
                nc.gpsimd.affine_select(
                    out=ident2_f32[:],
                    in_=ident2_f32[:],
                    compare_op=mybir.AluOpType.not_equal,
                    fill=1.0,
                    base=base,
                    pattern=[[-1, HD]],
                    channel_multiplier=1,
                )
            ident2 = const.tile([P, HD], MM_DT)
            nc.vector.tensor_copy(ident2[:], ident2_f32[:])
            ones1 = const.tile([1, P], MM_DT)
            nc.vector.memset(ones1[:].bitcast(f32), 1.0)
            # mask128[k, q] = 1.0 if k <= q else 0.0
            mask128 = const.tile([P, P], f32)
            nc.gpsimd.memset(mask128[:], 1.0)
            nc.gpsimd.affine_select(
                out=mask128[:],
                in_=mask128[:],
                compare_op=mybir.AluOpType.is_ge,
                fill=0.0,
                base=0,
                pattern=[[1, P]],
                channel_multiplier=-1,
            )

            pools = (xin_pool, xt_pool, qkvt_pool, vaug_pool, pt_pool,
                     atn_pool, out_pool, small_pool, ps_mm, ps_s, ps_o)
            consts = (wqkv_sb, wp_sb, bqkv_sb, ident_f32, ident2, mask128,
                      ones1, x_d, out_d)
            for it in range(iters):
                _emit_body(nc, tc, pools, consts, it)

    nc.compile()
    return nc


_CACHE = {}


def get_program(iters=1):
    if iters not in _CACHE:
        _CACHE[iters] = _build_program(iters)
    return _CACHE[iters]


def make_in_maps(hidden_states, c_attn_w, c_attn_b, c_proj_w):
    x = np.ascontiguousarray(
        np.asarray(hidden_states, dtype=np.float32).reshape(T, D)
    )
    wa = np.asarray(c_attn_w, dtype=np.float32)
    ba = np.asarray(c_attn_b, dtype=np.float32)
    wp = np.asarray(c_proj_w, dtype=np.float32)
    in_maps = []
    for c in range(N_CORES):
        lo, hi = c * P, (c + 1) * P
        w_qkv = np.ascontiguousarray(
            np.concatenate(
                [wa[:, lo:hi], wa[:, D + lo : D + hi], wa[:, 2 * D + lo : 2 * D + hi]],
                axis=1,
            )
        )
        b_qkv = np.ascontiguousarray(
            np.concatenate([ba[lo:hi], ba[D + lo : D + hi], ba[2 * D + lo : 2 * D + hi]])
        )
        w_proj = np.ascontiguousarray(wp[lo:hi, :])
        in_maps.append({"x": x, "w_qkv": w_qkv, "b_qkv": b_qkv, "w_proj": w_proj})
    return in_maps


def kernel(hidden_states, c_attn_w, c_attn_b, c_proj_w, c_proj_b):
    nc = get_program()
    in_maps = make_in_maps(hidden_states, c_attn_w, c_attn_b, c_proj_w)
    res = run_bass_kernel_spmd(nc, in_maps, list(range(N_CORES)))
    # unshard: row-parallel projection partials sum + bias
    acc = res.results[0]["out"]
    for c in range(1, N_CORES):
        acc = acc + res.results[c]["out"]
    acc = acc + np.asarray(c_proj_b, dtype=np.float32)[None, :]
    return acc.reshape(B, S, D).astype(np.float32)


if __name__ == "__main__":
    rng = np.random.default_rng(0)
    hs = rng.standard_normal((B, S, D), dtype=np.float32)
    wa = rng.standard_normal((D, 3 * D), dtype=np.float32) * 0.02
    ba = rng.standard_normal((3 * D,), dtype=np.float32) * 0.02
    wp = rng.standard_normal((D, D), dtype=np.float32) * 0.02
    bp = rng.standard_normal((D,), dtype=np.float32) * 0.02
    out = kernel(hs, wa, ba, wp, bp)
    print("out", out.shape, out.dtype, float(np.abs(out).max()))
